# revision 13
# baseline (speedup 1.0000x reference)
"""CogKR GNN message-passing kernel for Trainium2 (8 NeuronCores, Bass/Tile).

Strategy:
  * Host: sort edges by tail_node; split the tail-node (segment) space into 8
    contiguous ranges with balanced edge counts -> each core owns a disjoint
    set of segments and all edges pointing into them. Zero collectives.
  * Per core, edges are packed into uniform groups of 512 edge slots (4 tiles
    of 128) whose segments fit inside a 128-segment window -> per-group
    segment-sum via one-hot matmul accumulated in PSUM (exact).
  * All row gathers (node_hidden, entity, relation tables, h_new projections)
    are chunked indirect DMAs from bf16 tables to amortize SWDGE overhead.
"""

import math
import os
import sys

import numpy as np

for _p in ("/opt/trn_rl_repo",):
    if _p not in sys.path:
        sys.path.append(_p)

import ml_dtypes

import concourse.bass as bass
import concourse.tile as tile
from concourse import mybir

BF16 = mybir.dt.bfloat16
F32 = mybir.dt.float32
I32 = mybir.dt.int32
AF = mybir.ActivationFunctionType
ALU = mybir.AluOpType
bf16 = ml_dtypes.bfloat16

# Problem dims (fixed by the nn_CogKR problem)
E = 128
H = 256
Q = 256
B = 256
N_ENT = 200000
N_REL = 500
N_NODES = 100000
N_NEW = 50000
M = 200000
NCORES = 8

TG = 4          # tiles (of 128 edges) per segment group -> 512 edge slots
GCAP = TG * 128  # edge capacity per group
CH = 4          # edge tiles per gather chunk
CHS = 4         # segment tiles per gather chunk


# ----------------------------------------------------------------------------
# Host-side sharding
# ----------------------------------------------------------------------------

def _shard(inputs):
    tail = np.asarray(inputs["tail_node"])
    order = np.argsort(tail, kind="stable")
    counts = np.bincount(tail, minlength=N_NEW)
    cum = np.concatenate([[0], np.cumsum(counts)])  # edges before segment s

    targets = (np.arange(1, NCORES) * M) // NCORES
    seg_bounds = np.searchsorted(cum, targets)
    seg_starts = np.concatenate([[0], seg_bounds, [N_NEW]])

    cores = []
    for c in range(NCORES):
        s0, s1 = int(seg_starts[c]), int(seg_starts[c + 1])
        e0, e1 = int(cum[s0]), int(cum[s1])
        eidx = order[e0:e1]  # global edge ids, sorted by tail segment
        # pack segments [s0, s1) into groups: <=128 segs and <=GCAP edges
        groups = []  # (seg_lo, seg_hi, edge_lo, edge_hi) local edge offsets
        s = s0
        while s < s1:
            gs0 = s
            ecnt = 0
            while s < s1 and (s - gs0) < 128 and ecnt + counts[s] <= GCAP:
                ecnt += int(counts[s])
                s += 1
            if s == gs0:
                raise RuntimeError("segment with more than GCAP edges")
            groups.append((gs0, s, int(cum[gs0]) - e0, int(cum[s]) - e0))
        cores.append(dict(s0=s0, s1=s1, eidx=eidx, groups=groups))

    NG = max(len(cc["groups"]) for cc in cores)
    NG = ((NG + 1) // 2) * 2  # even -> NT multiple of CH
    NT = NG * TG
    LMAX = NT * 128
    nseg_max = max(cc["s1"] - cc["s0"] for cc in cores)
    NSB = ((nseg_max + 1 + CHS * 128 - 1) // (CHS * 128)) * CHS
    NSEG = NSB * 128

    shards = []
    for c, cc in enumerate(cores):
        s0 = cc["s0"]
        nseg = cc["s1"] - s0
        eidx = cc["eidx"]
        head = np.zeros(LMAX, np.int32)
        rel = np.zeros(LMAX, np.int32)
        qrel = np.zeros(LMAX, np.int32)
        tent = np.zeros(LMAX, np.int32)
        bidx = np.zeros(LMAX, np.int32)
        ihw = np.zeros(LMAX, np.int32)
        segrel = np.full(LMAX, -1.0e6, np.float32)
        gslot = np.full(LMAX, -1, np.int64)  # global edge id per slot
        iagg = np.zeros(NSEG, np.int32)
        itne = np.zeros(NSEG, np.int32)

        for g, (gs0, gs1, el0, el1) in enumerate(cc["groups"]):
            n = el1 - el0
            sl = g * GCAP
            ge = eidx[el0:el1]
            head[sl:sl + n] = inputs["head_node"][ge]
            rel[sl:sl + n] = inputs["edge_rel"][ge]
            qrel[sl:sl + n] = inputs["query_rel"][ge]
            tent[sl:sl + n] = inputs["tail_ent"][ge]
            bidx[sl:sl + n] = inputs["batch_idx"][ge]
            segs = tail[ge] - s0  # local segment id
            ihw[sl:sl + n] = segs
            segrel[sl:sl + n] = (segs - (gs0 - s0)).astype(np.float32)
            gslot[sl:sl + n] = ge
            # agg gather index for this group's segments
            ls = np.arange(gs0 - s0, gs1 - s0)
            iagg[ls] = g * 128 + (ls - (gs0 - s0))

        itne[:nseg] = inputs["tail_node_ent"][s0:s0 + nseg]

        def tiled(a, dt):
            return np.ascontiguousarray(
                a.reshape(-1, 128).T.astype(dt))  # [128, ntiles]

        shards.append(dict(
            i_head=tiled(head, np.int32),
            i_rel=tiled(rel, np.int32),
            i_qrel=tiled(qrel, np.int32),
            i_tent=tiled(tent, np.int32),
            i_bidx=tiled(bidx, np.int32),
            i_hw=tiled(ihw, np.int32),
            segrel=tiled(segrel, np.float32),
            i_agg=tiled(iagg, np.int32),
            i_tne=tiled(itne, np.int32),
            gslot=gslot,
        ))

    meta = dict(NG=NG, NT=NT, LMAX=LMAX, NSB=NSB, NSEG=NSEG)
    return shards, meta


# ----------------------------------------------------------------------------
# Device program
# ----------------------------------------------------------------------------

def _build(meta):
    NG, NT, NSB, NSEG = meta["NG"], meta["NT"], meta["NSB"], meta["NSEG"]
    nc = bass.Bass()

    def din(name, shape, dt):
        return nc.dram_tensor(name, shape, dt, kind="ExternalInput")

    # tables
    nodeh = din("nodeh", [N_NODES, H], BF16)
    ent = din("ent", [N_ENT, E], BF16)
    relpad = din("relpad", [512, E], BF16)
    queryr = din("queryr", [B, Q], F32)
    # weights (bf16 for matmul operands)
    ws_w = din("ws_w", [H, H], BF16)
    wr_w = din("wr_w", [E, H], BF16)
    wqr_w = din("wqr_w", [E, H], BF16)
    we2h_w = din("we2h_w", [E, H], BF16)
    wiht_w = din("wiht_w", [2 * E, 3 * H], BF16)
    whht_w = din("whht_w", [H, 3 * H], BF16)
    cw1_w = din("cw1_w", [E, H], BF16)
    cw2_w = din("cw2_w", [E, H], BF16)
    cw3_w = din("cw3_w", [H, H], BF16)
    # broadcast constants (f32)
    iota_b = din("iota_b", [128, 128], F32)
    walpha_b = din("walpha_b", [128, H], F32)
    rankwa_b = din("rankwa_b", [128, H], F32)
    rankwb_b = din("rankwb_b", [128, Q], F32)
    bqr_b = din("bqr_b", [128, H], F32)
    brz_b = din("brz_b", [128, 2 * H], F32)
    bnx_b = din("bnx_b", [128, H], F32)
    bnh_b = din("bnh_b", [128, H], F32)
    we2hb_b = din("we2hb_b", [128, H], F32)
    lng_b = din("lng_b", [128, H], F32)
    lnb_b = din("lnb_b", [128, H], F32)
    candb_b = din("candb_b", [128, H], F32)
    we2hbc = din("we2hbc", [128, 2], F32)
    balpha_c = din("balpha_c", [128, 1], F32)
    parange_c = din("parange_c", [128, 1], F32)
    rankb_c = din("rankb_c", [128, 1], F32)
    # per-core index arrays
    i_head = din("i_head", [128, NT], I32)
    i_rel = din("i_rel", [128, NT], I32)
    i_qrel = din("i_qrel", [128, NT], I32)
    i_tent = din("i_tent", [128, NT], I32)
    i_bidx = din("i_bidx", [128, NT], I32)
    i_hw = din("i_hw", [128, NT], I32)
    segrel_in = din("segrel", [128, NT], F32)
    i_agg = din("i_agg", [128, NSB], I32)
    i_tne = din("i_tne", [128, NSB], I32)

    scores_out = nc.dram_tensor("scores", [128, NT], F32, kind="ExternalOutput")

    # scratch DRAM
    relcat_d = nc.dram_tensor("relcat_d", [512, E + H], BF16)
    relwq_d = nc.dram_tensor("relwq_d", [512, H], BF16)
    relc_d = nc.dram_tensor("relc_d", [512, H], BF16)
    qrep_d = nc.dram_tensor("qrep_d", [B, 64], F32)
    groups_d = nc.dram_tensor("groups_d", [NG * 128, 2 * E], F32)
    hw_d = nc.dram_tensor("hw_d", [NSEG, H], BF16)

    with tile.TileContext(nc) as tc:
        with (
            tc.tile_pool(name="singles", bufs=1) as singles,
            tc.tile_pool(name="gath", bufs=2) as gath,
            tc.tile_pool(name="work", bufs=2) as work,
            tc.tile_pool(name="small", bufs=3) as small,
            tc.tile_pool(name="ps128", bufs=3, space="PSUM") as ps128,
            tc.tile_pool(name="ps256a", bufs=2, space="PSUM") as ps256a,
            tc.tile_pool(name="ps256b", bufs=2, space="PSUM") as ps256b,
            tc.tile_pool(name="ps512", bufs=1, space="PSUM") as ps512,
        ):
            # ---- load constants ----
            def load(t, shape, dt):
                s = singles.tile(shape, dt, tag=f"c_{t.name}")
                nc.sync.dma_start(out=s[:], in_=t[:])
                return s

            def loadk(t, rows, n, dt):
                c = rows // 128
                s = singles.tile([128, c, n], dt, tag=f"c_{t.name}")
                nc.sync.dma_start(
                    out=s[:], in_=t[:].rearrange("(c p) n -> p c n", p=128))
                return s

            ws_s = loadk(ws_w, H, H, BF16)
            wr_s = load(wr_w, [E, H], BF16)
            wqr_s = load(wqr_w, [E, H], BF16)
            we2h_s = load(we2h_w, [E, H], BF16)
            wiht_s = loadk(wiht_w, 2 * E, 3 * H, BF16)
            whht_s = loadk(whht_w, H, 3 * H, BF16)
            cw1_s = load(cw1_w, [E, H], BF16)
            cw2_s = load(cw2_w, [E, H], BF16)
            cw3_s = loadk(cw3_w, H, H, BF16)
            iota_s = load(iota_b, [128, 128], F32)
            walpha_s = load(walpha_b, [128, H], F32)
            rankwa_s = load(rankwa_b, [128, H], F32)
            rankwb_s = load(rankwb_b, [128, Q], F32)
            bqr_s = load(bqr_b, [128, H], F32)
            brz_s = load(brz_b, [128, 2 * H], F32)
            bnx_s = load(bnx_b, [128, H], F32)
            bnh_s = load(bnh_b, [128, H], F32)
            we2hb_s = load(we2hb_b, [128, H], F32)
            lng_s = load(lng_b, [128, H], F32)
            lnb_s = load(lnb_b, [128, H], F32)
            candb_s = load(candb_b, [128, H], F32)
            we2hbc_s = load(we2hbc, [128, 2], F32)
            balpha_s = load(balpha_c, [128, 1], F32)
            rankb_s = load(rankb_c, [128, 1], F32)
            ih_head = load(i_head, [128, NT], I32)
            ih_rel = load(i_rel, [128, NT], I32)
            ih_qrel = load(i_qrel, [128, NT], I32)
            ih_tent = load(i_tent, [128, NT], I32)
            ih_bidx = load(i_bidx, [128, NT], I32)
            ih_hw = load(i_hw, [128, NT], I32)
            segrel_s = load(segrel_in, [128, NT], F32)
            ih_agg = load(i_agg, [128, NSB], I32)
            ih_tne = load(i_tne, [128, NSB], I32)

            parange_s = load(parange_c, [128, 1], F32)
            ident_b = singles.tile([128, 128], BF16)
            nc.vector.tensor_scalar(
                out=ident_b[:], in0=iota_s[:], scalar1=parange_s[:],
                scalar2=None, op0=ALU.is_equal)
            ident_f = singles.tile([128, 128], F32)
            nc.vector.tensor_scalar(
                out=ident_f[:], in0=iota_s[:], scalar1=parange_s[:],
                scalar2=None, op0=ALU.is_equal)
            eps_s = singles.tile([128, 1], F32)
            nc.vector.memset(eps_s[:], 1e-5)

            scores_sb = singles.tile([128, NT], F32)

            # ---- stage 0: projected relation tables + qproj ----
            for r in range(4):
                rel_sb = small.tile([128, E], BF16, tag="rel_sb")
                nc.sync.dma_start(out=rel_sb[:], in_=relpad[r * 128:(r + 1) * 128, :])
                tp = ps128.tile([128, 128], BF16, tag="t128")
                nc.tensor.transpose(out=tp[:], in_=rel_sb[:], identity=ident_b[:])
                relT = small.tile([128, 128], BF16, tag="relT")
                nc.any.tensor_copy(out=relT[:], in_=tp[:])
                # raw relation rows -> relcat[:, 0:E]
                nc.sync.dma_start(out=relcat_d[r * 128:(r + 1) * 128, 0:E],
                                  in_=rel_sb[:])
                # relW' = rel @ Wr + bqr
                pw = ps256a.tile([128, H], F32, tag="t256a")
                nc.tensor.matmul(out=pw[:], lhsT=relT[:], rhs=wr_s[:],
                                 start=True, stop=True)
                relw_sb = small.tile([128, H], BF16, tag="relw_sb")
                nc.vector.tensor_tensor(out=relw_sb[:], in0=pw[:], in1=bqr_s[:],
                                        op=ALU.add)
                nc.sync.dma_start(out=relcat_d[r * 128:(r + 1) * 128, E:E + H],
                                  in_=relw_sb[:])
                # relWq = rel @ Wqr
                pq = ps256b.tile([128, H], F32, tag="t256b")
                nc.tensor.matmul(out=pq[:], lhsT=relT[:], rhs=wqr_s[:],
                                 start=True, stop=True)
                relq_sb = small.tile([128, H], BF16, tag="relq_sb")
                nc.any.tensor_copy(out=relq_sb[:], in_=pq[:])
                nc.sync.dma_start(out=relwq_d[r * 128:(r + 1) * 128, :],
                                  in_=relq_sb[:])
                # relC = rel @ cW2
                pc = ps256a.tile([128, H], F32, tag="t256a")
                nc.tensor.matmul(out=pc[:], lhsT=relT[:], rhs=cw2_s[:],
                                 start=True, stop=True)
                relc_sb = small.tile([128, H], BF16, tag="relc_sb")
                nc.any.tensor_copy(out=relc_sb[:], in_=pc[:])
                nc.sync.dma_start(out=relc_d[r * 128:(r + 1) * 128, :],
                                  in_=relc_sb[:])

            # qproj = query @ rank_Wb + rank_b, replicated x64
            for qb in range(B // 128):
                q_sb = small.tile([128, Q], F32, tag="q_sb")
                nc.sync.dma_start(out=q_sb[:], in_=queryr[qb * 128:(qb + 1) * 128, :])
                junk = small.tile([128, Q], F32, tag="q_junk")
                qp = small.tile([128, 1], F32, tag="qp")
                nc.vector.tensor_tensor(out=junk[:], in0=q_sb[:],
                                        in1=rankwb_s[:], op=ALU.mult)
                nc.vector.tensor_reduce(out=qp[:], in_=junk[:],
                                        axis=mybir.AxisListType.X, op=ALU.add)
                nc.vector.tensor_tensor(out=qp[:], in0=qp[:], in1=rankb_s[:],
                                        op=ALU.add)
                qrep_sb = small.tile([128, 64], F32, tag="qrep_sb")
                nc.any.tensor_copy(out=qrep_sb[:], in_=qp[:].to_broadcast([128, 64]))
                nc.sync.dma_start(out=qrep_d[qb * 128:(qb + 1) * 128, :],
                                  in_=qrep_sb[:])

            # ---- pass A: per-edge attention, messages, segment sums ----
            NCHUNKS = NT // CH
            te_all = singles.tile([128, NT, E], BF16)
            for ch in range(NCHUNKS):
                t0 = ch * CH
                hs_g = gath.tile([128, CH, H], BF16, tag="hs_g")
                rc_g = gath.tile([128, CH, E + H], BF16, tag="rc_g")
                wq_g = gath.tile([128, CH, H], BF16, tag="wq_g")
                for j in range(CH):
                    t = t0 + j
                    nc.gpsimd.indirect_dma_start(
                        out=hs_g[:, j, :], out_offset=None, in_=nodeh[:],
                        in_offset=bass.IndirectOffsetOnAxis(
                            ap=ih_head[:, t:t + 1], axis=0))
                    nc.gpsimd.indirect_dma_start(
                        out=rc_g[:, j, :], out_offset=None, in_=relcat_d[:],
                        in_offset=bass.IndirectOffsetOnAxis(
                            ap=ih_rel[:, t:t + 1], axis=0))
                    nc.gpsimd.indirect_dma_start(
                        out=wq_g[:, j, :], out_offset=None, in_=relwq_d[:],
                        in_offset=bass.IndirectOffsetOnAxis(
                            ap=ih_qrel[:, t:t + 1], axis=0))
                    nc.gpsimd.indirect_dma_start(
                        out=te_all[:, t, :], out_offset=None, in_=ent[:],
                        in_offset=bass.IndirectOffsetOnAxis(
                            ap=ih_tent[:, t:t + 1], axis=0))

                s1 = work.tile([128, CH, H], BF16, tag="s1")
                nc.any.tensor_tensor(out=s1[:], in0=rc_g[:, :, E:E + H],
                                     in1=wq_g[:], op=ALU.add)
                attn = work.tile([128, CH, H], BF16, tag="attn")
                for j in range(CH):
                    hst = work.tile([128, 2, 128], BF16, tag="hst")
                    for hc in range(2):
                        tp = ps128.tile([128, 128], BF16, tag="t128")
                        nc.tensor.transpose(
                            out=tp[:], in_=hs_g[:, j, hc * 128:(hc + 1) * 128],
                            identity=ident_b[:])
                        nc.any.tensor_copy(out=hst[:, hc, :], in_=tp[:])
                    pa = ps256a.tile([128, H], F32, tag="t256a")
                    nc.tensor.matmul(out=pa[:], lhsT=hst[:, 0, :],
                                     rhs=ws_s[:, 0, :], start=True, stop=False)
                    nc.tensor.matmul(out=pa[:], lhsT=hst[:, 1, :],
                                     rhs=ws_s[:, 1, :], start=False, stop=True)
                    nc.vector.tensor_tensor(out=attn[:, j, :], in0=pa[:],
                                            in1=s1[:, j, :], op=ALU.add)
                    nc.scalar.activation(out=attn[:, j, :], in_=attn[:, j, :],
                                         func=AF.Relu)
                # alpha = sigmoid(attn @ w_alpha + b_alpha) per tile
                am = work.tile([128, CH, H], BF16, tag="am")
                nc.vector.tensor_tensor(
                    out=am[:], in0=attn[:],
                    in1=walpha_s[:, None, :].to_broadcast([128, CH, H]),
                    op=ALU.mult)
                alpha = small.tile([128, CH], F32, tag="alpha")
                nc.vector.tensor_reduce(out=alpha[:], in_=am[:],
                                        axis=mybir.AxisListType.X, op=ALU.add)
                nc.scalar.activation(out=alpha[:], in_=alpha[:], func=AF.Sigmoid,
                                     bias=balpha_s[:], scale=1.0)
                # msg = [hr, te] * alpha
                msg = work.tile([128, CH, 2 * E], BF16, tag="msg")
                ab = alpha[:, :, None].to_broadcast([128, CH, E])
                nc.any.tensor_tensor(out=msg[:, :, 0:E], in0=rc_g[:, :, 0:E],
                                     in1=ab, op=ALU.mult)
                nc.any.tensor_tensor(out=msg[:, :, E:2 * E],
                                     in0=te_all[:, t0:t0 + CH, :],
                                     in1=ab, op=ALU.mult)
                # segment sums: one-hot matmul per group
                for gi in range(CH // TG):
                    g = ch * (CH // TG) + gi
                    pg = ps256b.tile([128, 2 * E], F32, tag="t256b")
                    for jj in range(TG):
                        j = gi * TG + jj
                        t = t0 + j
                        oh = small.tile([128, 128], BF16, tag="oh")
                        nc.any.tensor_scalar(
                            out=oh[:], in0=iota_s[:],
                            scalar1=segrel_s[:, t:t + 1], scalar2=None,
                            op0=ALU.is_equal)
                        nc.tensor.matmul(out=pg[:], lhsT=oh[:], rhs=msg[:, j, :],
                                         start=(jj == 0), stop=(jj == TG - 1))
                    gout = small.tile([128, 2 * E], F32, tag="gout")
                    nc.any.tensor_copy(out=gout[:], in_=pg[:])
                    nc.sync.dma_start(out=groups_d[g * 128:(g + 1) * 128, :],
                                      in_=gout[:])

            # ---- GRU + LayerNorm + cand-projection per segment tile ----
            for ch in range(NSB // CHS):
                j0 = ch * CHS
                agg_g = gath.tile([128, CHS, 2 * E], F32, tag="agg_g")
                eg = gath.tile([128, CHS, E], BF16, tag="eg")
                for j in range(CHS):
                    nc.gpsimd.indirect_dma_start(
                        out=agg_g[:, j, :], out_offset=None, in_=groups_d[:],
                        in_offset=bass.IndirectOffsetOnAxis(
                            ap=ih_agg[:, j0 + j:j0 + j + 1], axis=0))
                    nc.gpsimd.indirect_dma_start(
                        out=eg[:, j, :], out_offset=None, in_=ent[:],
                        in_offset=bass.IndirectOffsetOnAxis(
                            ap=ih_tne[:, j0 + j:j0 + j + 1], axis=0))
                for j in range(CHS):
                    sj = j0 + j
                    # transposes
                    aggt = work.tile([128, 2, 128], BF16, tag="aggt")
                    for hc in range(2):
                        tp = ps128.tile([128, 128], F32, tag="t128")
                        nc.tensor.transpose(
                            out=tp[:], in_=agg_g[:, j, hc * 128:(hc + 1) * 128],
                            identity=ident_f[:])
                        nc.any.tensor_copy(out=aggt[:, hc, :], in_=tp[:])
                    tpe = ps128.tile([128, 128], BF16, tag="t128")
                    nc.tensor.transpose(out=tpe[:], in_=eg[:, j, :],
                                        identity=ident_b[:])
                    et = work.tile([128, 128], BF16, tag="et")
                    nc.any.tensor_copy(out=et[:], in_=tpe[:])
                    # h_prev transposed [h, seg] (lrelu with per-partition bias)
                    hpt = work.tile([128, 2, 128], BF16, tag="hpt")
                    for hc in range(2):
                        php = ps128.tile([128, 128], F32, tag="t128")
                        nc.tensor.matmul(out=php[:],
                                         lhsT=we2h_s[:, hc * 128:(hc + 1) * 128],
                                         rhs=et[:], start=True, stop=True)
                        hpf = work.tile([128, 128], F32, tag="hpf")
                        nc.vector.tensor_scalar(
                            out=hpf[:], in0=php[:],
                            scalar1=we2hbc_s[:, hc:hc + 1], scalar2=None,
                            op0=ALU.add)
                        hps = work.tile([128, 128], F32, tag="hps")
                        nc.any.tensor_scalar(
                            out=hps[:], in0=hpf[:], scalar1=0.01, scalar2=None,
                            op0=ALU.mult)
                        nc.any.tensor_tensor(out=hpt[:, hc, :], in0=hpf[:],
                                             in1=hps[:], op=ALU.max)
                    # h_prev normal [seg, h]
                    php2 = ps256a.tile([128, H], F32, tag="t256a")
                    nc.tensor.matmul(out=php2[:], lhsT=et[:], rhs=we2h_s[:],
                                     start=True, stop=True)
                    hp = work.tile([128, H], F32, tag="hp")
                    nc.vector.tensor_tensor(out=hp[:], in0=php2[:], in1=we2hb_s[:],
                                            op=ALU.add)
                    hp2 = work.tile([128, H], F32, tag="hp2")
                    nc.any.tensor_scalar(out=hp2[:], in0=hp[:], scalar1=0.01,
                                         scalar2=None, op0=ALU.mult)
                    nc.any.tensor_tensor(out=hp[:], in0=hp[:], in1=hp2[:],
                                         op=ALU.max)
                    # gates
                    prz = ps512.tile([128, 2 * H], F32, tag="t512")
                    nc.tensor.matmul(out=prz[:], lhsT=aggt[:, 0, :],
                                     rhs=wiht_s[:, 0, 0:2 * H], start=True, stop=False)
                    nc.tensor.matmul(out=prz[:], lhsT=aggt[:, 1, :],
                                     rhs=wiht_s[:, 1, 0:2 * H], start=False, stop=False)
                    nc.tensor.matmul(out=prz[:], lhsT=hpt[:, 0, :],
                                     rhs=whht_s[:, 0, 0:2 * H], start=False, stop=False)
                    nc.tensor.matmul(out=prz[:], lhsT=hpt[:, 1, :],
                                     rhs=whht_s[:, 1, 0:2 * H], start=False, stop=True)
                    pxn = ps256a.tile([128, H], F32, tag="t256a")
                    nc.tensor.matmul(out=pxn[:], lhsT=aggt[:, 0, :],
                                     rhs=wiht_s[:, 0, 2 * H:3 * H], start=True, stop=False)
                    nc.tensor.matmul(out=pxn[:], lhsT=aggt[:, 1, :],
                                     rhs=wiht_s[:, 1, 2 * H:3 * H], start=False, stop=True)
                    phn = ps256b.tile([128, H], F32, tag="t256b")
                    nc.tensor.matmul(out=phn[:], lhsT=hpt[:, 0, :],
                                     rhs=whht_s[:, 0, 2 * H:3 * H], start=True, stop=False)
                    nc.tensor.matmul(out=phn[:], lhsT=hpt[:, 1, :],
                                     rhs=whht_s[:, 1, 2 * H:3 * H], start=False, stop=True)
                    rz = work.tile([128, 2 * H], F32, tag="rz")
                    nc.vector.tensor_tensor(out=rz[:], in0=prz[:], in1=brz_s[:],
                                            op=ALU.add)
                    nc.scalar.activation(out=rz[:], in_=rz[:], func=AF.Sigmoid)
                    hn2 = work.tile([128, H], F32, tag="hn2")
                    nc.vector.tensor_tensor(out=hn2[:], in0=phn[:], in1=bnh_s[:],
                                            op=ALU.add)
                    nc.any.tensor_tensor(out=hn2[:], in0=rz[:, 0:H], in1=hn2[:],
                                         op=ALU.mult)
                    npre = work.tile([128, H], F32, tag="npre")
                    nc.vector.tensor_tensor(out=npre[:], in0=pxn[:], in1=bnx_s[:],
                                            op=ALU.add)
                    nc.any.tensor_tensor(out=npre[:], in0=npre[:], in1=hn2[:],
                                         op=ALU.add)
                    nc.scalar.activation(out=npre[:], in_=npre[:], func=AF.Tanh)
                    # h_new = n + z*(h_prev - n)
                    hnew = work.tile([128, H], F32, tag="hnew")
                    nc.vector.tensor_tensor(out=hnew[:], in0=hp[:], in1=npre[:],
                                            op=ALU.subtract)
                    nc.any.tensor_tensor(out=hnew[:], in0=rz[:, H:2 * H],
                                         in1=hnew[:], op=ALU.mult)
                    nc.vector.tensor_tensor(out=hnew[:], in0=npre[:], in1=hnew[:],
                                            op=ALU.add)
                    # LayerNorm
                    stats = small.tile([128, 6], F32, tag="stats")
                    nc.vector.bn_stats(out=stats[:], in_=hnew[:])
                    mv = small.tile([128, 2], F32, tag="mv")
                    nc.vector.bn_aggr(out=mv[:], in_=stats[:])
                    rstd = small.tile([128, 1], F32, tag="rstd")
                    nc.scalar.activation(out=rstd[:], in_=mv[:, 1:2], func=AF.Sqrt,
                                         bias=eps_s[:], scale=1.0)
                    nc.vector.reciprocal(out=rstd[:], in_=rstd[:])
                    hln = work.tile([128, H], BF16, tag="hln")
                    nc.vector.tensor_scalar(
                        out=hnew[:], in0=hnew[:], scalar1=mv[:, 0:1],
                        scalar2=rstd[:], op0=ALU.subtract, op1=ALU.mult)
                    nc.any.tensor_tensor(out=hnew[:], in0=hnew[:], in1=lng_s[:],
                                         op=ALU.mult)
                    nc.any.tensor_tensor(out=hln[:], in0=hnew[:], in1=lnb_s[:],
                                         op=ALU.add)
                    # hW = h_new @ cW3 + cand_b  (for pass B gathering)
                    hlt = work.tile([128, 2, 128], BF16, tag="hlt")
                    for hc in range(2):
                        tp2 = ps128.tile([128, 128], BF16, tag="t128")
                        nc.tensor.transpose(
                            out=tp2[:], in_=hln[:, hc * 128:(hc + 1) * 128],
                            identity=ident_b[:])
                        nc.any.tensor_copy(out=hlt[:, hc, :], in_=tp2[:])
                    phw = ps256a.tile([128, H], F32, tag="t256a")
                    nc.tensor.matmul(out=phw[:], lhsT=hlt[:, 0, :],
                                     rhs=cw3_s[:, 0, :], start=True, stop=False)
                    nc.tensor.matmul(out=phw[:], lhsT=hlt[:, 1, :],
                                     rhs=cw3_s[:, 1, :], start=False, stop=True)
                    hw_sb = work.tile([128, H], BF16, tag="hw_sb")
                    nc.vector.tensor_tensor(out=hw_sb[:], in0=phw[:], in1=candb_s[:],
                                            op=ALU.add)
                    nc.sync.dma_start(out=hw_d[sj * 128:(sj + 1) * 128, :],
                                      in_=hw_sb[:])

            # ---- pass B: candidate scoring ----
            for ch in range(NCHUNKS):
                t0 = ch * CH
                rc2 = gath.tile([128, CH, H], BF16, tag="rc2")
                hw_g = gath.tile([128, CH, H], BF16, tag="hw_g")
                qr_g = gath.tile([128, CH, 64], F32, tag="qr_g")
                for j in range(CH):
                    t = t0 + j
                    nc.gpsimd.indirect_dma_start(
                        out=rc2[:, j, :], out_offset=None, in_=relc_d[:],
                        in_offset=bass.IndirectOffsetOnAxis(
                            ap=ih_rel[:, t:t + 1], axis=0))
                    nc.gpsimd.indirect_dma_start(
                        out=hw_g[:, j, :], out_offset=None, in_=hw_d[:],
                        in_offset=bass.IndirectOffsetOnAxis(
                            ap=ih_hw[:, t:t + 1], axis=0))
                    nc.gpsimd.indirect_dma_start(
                        out=qr_g[:, j, :], out_offset=None, in_=qrep_d[:],
                        in_offset=bass.IndirectOffsetOnAxis(
                            ap=ih_bidx[:, t:t + 1], axis=0))
                s2 = work.tile([128, CH, H], BF16, tag="s2")
                nc.any.tensor_tensor(out=s2[:], in0=rc2[:], in1=hw_g[:], op=ALU.add)
                for j in range(CH):
                    t = t0 + j
                    tp3 = ps128.tile([128, 128], BF16, tag="t128")
                    nc.tensor.transpose(out=tp3[:], in_=te_all[:, t, :],
                                        identity=ident_b[:])
                    tet = work.tile([128, 128], BF16, tag="tet")
                    nc.any.tensor_copy(out=tet[:], in_=tp3[:])
                    pcand = ps256a.tile([128, H], F32, tag="t256a")
                    nc.tensor.matmul(out=pcand[:], lhsT=tet[:], rhs=cw1_s[:],
                                     start=True, stop=True)
                    cand = work.tile([128, H], F32, tag="cand")
                    nc.vector.tensor_tensor(out=cand[:], in0=pcand[:],
                                            in1=s2[:, j, :], op=ALU.add)
                    cnd2 = work.tile([128, H], F32, tag="cnd2")
                    nc.any.tensor_scalar(out=cnd2[:], in0=cand[:], scalar1=0.01,
                                         scalar2=None, op0=ALU.mult)
                    nc.any.tensor_tensor(out=cand[:], in0=cand[:], in1=cnd2[:],
                                         op=ALU.max)
                    junk2 = work.tile([128, H], F32, tag="junk2")
                    rsum = small.tile([128, 1], F32, tag="rsum")
                    nc.vector.tensor_tensor(out=junk2[:], in0=cand[:],
                                            in1=rankwa_s[:], op=ALU.mult)
                    nc.vector.tensor_reduce(out=rsum[:], in_=junk2[:],
                                            axis=mybir.AxisListType.X, op=ALU.add)
                    nc.any.tensor_tensor(out=scores_sb[:, t:t + 1], in0=rsum[:],
                                         in1=qr_g[:, j, 0:1], op=ALU.add)

            nc.sync.dma_start(out=scores_out[:], in_=scores_sb[:])

    return nc


# ----------------------------------------------------------------------------
# Host wrapper
# ----------------------------------------------------------------------------

def _in_maps(inputs, shards, meta):
    f = np.float32
    ones_col = np.ones((128, 1), f)

    def bc(v, n):  # broadcast row vector to [128, n]
        return np.ascontiguousarray(np.broadcast_to(
            np.asarray(v, f).reshape(1, n), (128, n)))

    relpad = np.zeros((512, E), bf16)
    relpad[:N_REL] = np.asarray(inputs["relation_emb"], f).astype(bf16)
    W_ih = np.asarray(inputs["W_ih"], f)
    W_hh = np.asarray(inputs["W_hh"], f)
    cand_W = np.asarray(inputs["cand_W"], f)
    rank_W = np.asarray(inputs["rank_W"], f).reshape(-1)
    b_ih = np.asarray(inputs["b_ih"], f)
    b_hh = np.asarray(inputs["b_hh"], f)

    common = dict(
        nodeh=np.asarray(inputs["node_hidden"], f).astype(bf16),
        ent=np.asarray(inputs["entity_emb"], f).astype(bf16),
        relpad=relpad,
        queryr=np.asarray(inputs["query_repr"], f),
        ws_w=np.asarray(inputs["Ws"], f).astype(bf16),
        wr_w=np.asarray(inputs["Wr"], f).astype(bf16),
        wqr_w=np.asarray(inputs["Wqr"], f).astype(bf16),
        we2h_w=np.asarray(inputs["We2h_W"], f).astype(bf16),
        wiht_w=np.ascontiguousarray(W_ih.T).astype(bf16),
        whht_w=np.ascontiguousarray(W_hh.T).astype(bf16),
        cw1_w=np.ascontiguousarray(cand_W[0:E]).astype(bf16),
        cw2_w=np.ascontiguousarray(cand_W[E:2 * E]).astype(bf16),
        cw3_w=np.ascontiguousarray(cand_W[2 * E:]).astype(bf16),
        iota_b=bc(np.arange(128), 128),
        walpha_b=bc(np.asarray(inputs["w_alpha"], f).reshape(-1), H),
        rankwa_b=bc(rank_W[:H], H),
        rankwb_b=bc(rank_W[H:], Q),
        bqr_b=bc(inputs["bqr"], H),
        brz_b=bc(b_ih[:2 * H] + b_hh[:2 * H], 2 * H),
        bnx_b=bc(b_ih[2 * H:], H),
        bnh_b=bc(b_hh[2 * H:], H),
        we2hb_b=bc(inputs["We2h_b"], H),
        we2hbc=np.ascontiguousarray(
            np.asarray(inputs["We2h_b"], f).reshape(2, 128).T),
        lng_b=bc(inputs["ln_g"], H),
        lnb_b=bc(inputs["ln_b"], H),
        candb_b=bc(inputs["cand_b"], H),
        parange_c=np.arange(128, dtype=f).reshape(128, 1),
        balpha_c=np.ascontiguousarray(
            ones_col * np.asarray(inputs["b_alpha"], f).reshape(1, 1)),
        rankb_c=np.ascontiguousarray(
            ones_col * np.asarray(inputs["rank_b"], f).reshape(1, 1)),
    )
    maps = []
    for sh in shards:
        m = dict(common)
        for k in ("i_head", "i_rel", "i_qrel", "i_tent", "i_bidx", "i_hw",
                  "i_agg", "i_tne"):
            m[k] = sh[k]
        m["segrel"] = sh["segrel"]
        maps.append(m)
    return maps


def _unshard(results, shards, meta):
    scores = np.zeros(M, np.float32)
    for res, sh in zip(results, shards):
        out = np.asarray(res["scores"])  # [128, NT]
        flat = out.T.reshape(-1)  # slot-ordered
        valid = sh["gslot"] >= 0
        scores[sh["gslot"][valid]] = flat[valid]
    return scores


_PATCHED = False


def _split_multiwaits(raw: bytes) -> bytes:
    """Walrus in this container encodes at most one sem wait per instruction;
    Tile emits instructions with several. Split extras into standalone
    EventSemaphore waits on the same engine, just before the instruction."""
    import orjson
    j = orjson.loads(raw)
    n = 0
    for fn in j.get("functions", []):
        for bb in fn.get("blocks", []):
            out = []
            for ins in bb.get("instructions", []):
                si = ins.get("sync_info")
                ow = (si or {}).get("on_wait") or []
                if len(ow) > 1:
                    for k, w in enumerate(ow[:-1]):
                        out.append({
                            "debug": ins.get("debug", 0),
                            "engine": ins["engine"],
                            "ins": [], "outs": [],
                            "name": f"{ins['name']}_xw{k}",
                            "opcode": "EventSemaphore",
                            "sync_info": {"on_update": [], "on_wait": [w]},
                        })
                        n += 1
                    si["on_wait"] = [ow[-1]]
                out.append(ins)
            bb["instructions"] = out
    return orjson.dumps(j)


def _install_bir_patch():
    global _PATCHED
    if _PATCHED:
        return
    from concourse import bass2jax
    orig = bass2jax._decompress_ant_bir

    def patched(v):
        return _split_multiwaits(orig(v))

    bass2jax._decompress_ant_bir = patched
    _PATCHED = True


def _run_timed(nc, in_maps, n_cores, iters=6):
    """Execute the compiled kernel repeatedly; report steady-state wall time."""
    import time
    import jax
    import jax.numpy as jnp
    from jax.sharding import Mesh, PartitionSpec
    from jax.experimental.shard_map import shard_map
    from concourse import mybir as _mb
    from concourse.bass2jax import _bass_exec_p, install_neuronx_cc_hook, partition_id_tensor

    install_neuronx_cc_hook()
    partition_name = nc.partition_id_tensor.name if nc.partition_id_tensor else None
    in_names, out_names, out_avals = [], [], []
    zero_outs = []
    for alloc in nc.m.functions[0].allocations:
        if not isinstance(alloc, _mb.MemoryLocationSet):
            continue
        name = alloc.memorylocations[0].name
        if alloc.kind == "ExternalInput":
            if name != partition_name:
                in_names.append(name)
        elif alloc.kind == "ExternalOutput":
            out_names.append(name)
            shape = tuple(alloc.tensor_shape)
            dtype = _mb.dt.np(alloc.dtype)
            out_avals.append(jax.core.ShapedArray(shape, dtype))
            zero_outs.append(np.zeros(shape, dtype))
    n_params = len(in_names)
    in_names_all = in_names + out_names
    if partition_name is not None:
        in_names_all = in_names_all + [partition_name]

    def _body(*args):
        operands = list(args)
        if partition_name is not None:
            operands.append(partition_id_tensor())
        return tuple(_bass_exec_p.bind(
            *operands, out_avals=tuple(out_avals),
            in_names=tuple(in_names_all), out_names=tuple(out_names),
            lowering_input_output_aliases=(),
            sim_require_finite=True, sim_require_nnan=True, nc=nc))

    devices = jax.devices()[:n_cores]
    mesh = Mesh(np.asarray(devices), ("core",))
    in_specs = (PartitionSpec("core"),) * (n_params + len(out_names))
    out_specs = (PartitionSpec("core"),) * len(out_names)
    fn = jax.jit(shard_map(_body, mesh=mesh, in_specs=in_specs,
                           out_specs=out_specs, check_rep=False),
                 keep_unused=True)
    per_core = [[np.asarray(m[name]) for name in in_names] for m in in_maps]
    concat_in = [np.concatenate([per_core[c][i] for c in range(n_cores)], axis=0)
                 for i in range(n_params)]
    concat_zero = [np.concatenate([z] * n_cores, axis=0) for z in zero_outs]
    args = [jax.device_put(a) for a in concat_in + concat_zero]
    times = []
    outs = None
    for it in range(iters):
        t0 = time.time()
        outs = fn(*args)
        jax.block_until_ready(outs)
        times.append(time.time() - t0)
    best = min(times[1:]) if len(times) > 1 else times[0]
    print("exec wall times (s):", [f"{t:.4f}" for t in times])
    print(f"HW exec time: {best * 1e9 / 1:.0f} ns (all-8-core wall, min of steady)")
    res = []
    for c in range(n_cores):
        d = {}
        for i, name in enumerate(out_names):
            full = np.asarray(outs[i])
            percore = full.reshape(n_cores, -1, *full.shape[1:])[c] if False else \
                np.split(full, n_cores, axis=0)[c]
            d[name] = percore
        res.append(d)
    return res


def kernel(**inputs):
    shards, meta = _shard(inputs)
    nc = _build(meta)
    maps = _in_maps(inputs, shards, meta)

    if os.environ.get("BASS_KERNEL_SIM"):
        from concourse.bass_interp import CoreSim
        ncores = int(os.environ.get("BASS_KERNEL_SIM_CORES", "1"))
        results = []
        for c in range(ncores):
            sim = CoreSim(nc, require_finite=False, require_nnan=False)
            for k, v in maps[c].items():
                sim.tensor(k)[:] = v
            sim.simulate()
            results.append({"scores": np.array(sim.tensor("scores"))})
        # fill remaining cores with zeros so unshard works
        for c in range(ncores, NCORES):
            results.append({"scores": np.zeros((128, meta["NT"]), np.float32)})
        return _unshard(results, shards, meta)

    _install_bir_patch()
    if os.environ.get("BASS_KERNEL_TIME"):
        results = _run_timed(nc, maps, NCORES,
                             iters=int(os.environ.get("BASS_KERNEL_TIME", "6")))
        return _unshard(results, shards, meta)
    from concourse.bass_utils import run_bass_kernel_spmd
    res = run_bass_kernel_spmd(nc, maps, list(range(NCORES)))
    return _unshard(res.results, shards, meta)


if __name__ == "__main__":
    pass
